# revision 5
# baseline (speedup 1.0000x reference)
"""BitLinear (ternary-weight linear) Trainium2 kernel, 8-way tensor-parallel.

Computes reference:
    s   = max(mean(|W|), 1e-5)           (global scalar over the full weight)
    Wq  = clip(round(W / s), -1, 1)      (ternary {-1, 0, 1})
    xs  = x / max(|x|.max(-1), eps)      (per-token scaling)
    out = (xs @ Wq.T) * x_scale

Since the per-token activation scaling divides and then multiplies back the
exact same per-row scalar, out == x @ Wq.T up to fp32 rounding; the kernel
computes that directly.

Sharding: weight rows (out_features) split over 8 cores; x replicated.
Both operands are fed pre-transposed (K-major) so the contraction dim lands
on SBUF partitions without any on-device transposes:
    xt [K, M]      = x.reshape(M, K).T   (replicated)
    wt [K, N/8]    = W.T column shard
Per core: quantize its weight shard (the global scale comes from a scalar
AllReduce of per-shard |W| sums), then out_shard[M, N/8] = xt.T @ Wq via
bf16 matmuls with fp32 PSUM accumulation.
"""

import functools
import os
import sys

for _p in ("/opt/trn_rl_repo", os.path.expanduser("~/.axon_site/_ro/trn_rl_repo")):
    if os.path.isdir(_p) and _p not in sys.path:
        sys.path.append(_p)

from contextlib import ExitStack

import numpy as np

import concourse.bass as bass  # noqa: F401  (bass types used via bacc/tile)
import concourse.mybir as mybir
import concourse.tile as tile
from concourse import bacc
from concourse.bass_utils import run_bass_kernel_spmd

N_CORES = 8
B, S, K = 2, 4096, 4096
M = B * S                  # 8192 tokens
N = 16384                  # out_features
NS = N // N_CORES          # 2048 out_features per core
P = 128
KO = K // P                # 32 k-subtiles
MT = M // P                # 64 m-tiles
NT = NS // 512             # 4 n-tiles of 512
EPS = 1e-5

F32 = mybir.dt.float32
BF16 = mybir.dt.bfloat16

# Stash of the last BassKernelResults (for the dev harness to read timings).
LAST_RESULTS = None


def _build(nc=None):
    nc = bacc.Bacc(None, target_bir_lowering=False, num_devices=N_CORES)

    xt = nc.dram_tensor("xt", [K, M], F32, kind="ExternalInput")
    wt = nc.dram_tensor("wt", [K, NS], F32, kind="ExternalInput")
    out = nc.dram_tensor("out", [M, NS], F32, kind="ExternalOutput")

    xt_r = xt.rearrange("(ko p) m -> p ko m", p=P)     # [128, 32, 8192]
    wt_r = wt.rearrange("(ko p) n -> p ko n", p=P)     # [128, 32, 2048]
    out_r = out.rearrange("(mo p) n -> p mo n", p=P)   # [128, 64, 2048]

    with tile.TileContext(nc) as tc, ExitStack() as ctx:
        const = ctx.enter_context(tc.tile_pool(name="const", bufs=1))
        wqp = ctx.enter_context(tc.tile_pool(name="wqp", bufs=1))
        wstage = ctx.enter_context(tc.tile_pool(name="wstage", bufs=2))
        tmp = ctx.enter_context(tc.tile_pool(name="tmp", bufs=1))
        xstage = ctx.enter_context(tc.tile_pool(name="xstage", bufs=2))
        xbfp = ctx.enter_context(tc.tile_pool(name="xbfp", bufs=2))
        outp = ctx.enter_context(tc.tile_pool(name="outp", bufs=3))
        psum = ctx.enter_context(tc.tile_pool(name="psum", bufs=2, space="PSUM"))
        dram = ctx.enter_context(tc.tile_pool(name="dram", bufs=1, space="DRAM"))

        # ---- pass 1: per-shard sum of |w| -------------------------------
        partials = const.tile([P, KO], F32)
        for ko in range(KO):
            wst = wstage.tile([P, NS], F32, tag="wst")
            nc.sync.dma_start(wst[:], wt_r[:, ko, :])
            nc.vector.tensor_reduce(
                partials[:, ko : ko + 1], wst[:],
                axis=mybir.AxisListType.X, op=mybir.AluOpType.add,
                apply_absolute_value=True,
            )
        ptot = const.tile([P, 1], F32)
        nc.vector.tensor_reduce(
            ptot[:], partials[:], axis=mybir.AxisListType.X, op=mybir.AluOpType.add
        )
        ones = const.tile([P, 1], F32)
        nc.vector.memset(ones[:], 1.0)
        ssum_ps = psum.tile([1, 1], F32, tag="ps0")
        nc.tensor.matmul(ssum_ps[:], ones[:], ptot[:], start=True, stop=True)
        ssum = const.tile([1, 1], F32)
        nc.scalar.copy(ssum[:], ssum_ps[:])

        # ---- global scale via scalar AllReduce --------------------------
        cc_in = dram.tile([1, 1], F32)
        cc_out = dram.tile([1, 1], F32)
        nc.sync.dma_start(cc_in[:], ssum[:])
        nc.gpsimd.collective_compute(
            "AllReduce",
            mybir.AluOpType.add,
            replica_groups=[list(range(N_CORES))],
            ins=[cc_in.opt()],
            outs=[cc_out.opt()],
        )
        # thr = 0.5 * max(total/(N*K), EPS); note 0.5/(N*K) == 2^-27 exactly
        thr = const.tile([P, 1], F32)
        nc.sync.dma_start(thr[:], cc_out[:].to_broadcast((P, 1)))
        nc.vector.tensor_scalar(
            thr[:], thr[:], 0.5 / (N * K), 0.5 * EPS,
            mybir.AluOpType.mult, mybir.AluOpType.max,
        )
        nthr = const.tile([P, 1], F32)
        nc.vector.tensor_scalar(
            nthr[:], thr[:], -1.0, None, mybir.AluOpType.mult
        )

        # ---- pass 2: ternarize weight shard into resident bf16 ----------
        # wq = (w >= thr) - (w <= -thr)  in {-1, 0, 1}
        wq = wqp.tile([P, KO, NS], BF16)
        for ko in range(KO):
            wst = wstage.tile([P, NS], F32, tag="wst")
            nc.sync.dma_start(wst[:], wt_r[:, ko, :])
            t2 = tmp.tile([P, NS], F32, tag="t2")
            nc.vector.tensor_scalar(
                t2[:], wst[:], nthr[:], None, mybir.AluOpType.is_le
            )
            nc.vector.scalar_tensor_tensor(
                wq[:, ko, :], wst[:], thr[:], t2[:],
                mybir.AluOpType.is_ge, mybir.AluOpType.subtract,
            )

        # ---- matmul: out[m, n] = sum_k x[m, k] * wq[n, k] ----------------
        KC = 8  # f32 x staging chunk, in units of ko
        for mt in range(MT):
            xbf = xbfp.tile([P, KO, P], BF16, tag="xbf")
            for kc in range(0, KO, KC):
                xst = xstage.tile([P, KC, P], F32, tag="xst")
                nc.sync.dma_start(
                    xst[:], xt_r[:, kc : kc + KC, mt * P : (mt + 1) * P]
                )
                nc.scalar.copy(xbf[:, kc : kc + KC, :], xst[:])
            pss = [
                psum.tile([P, 512], F32, tag=f"ps{nt}", name=f"ps_{mt}_{nt}")
                for nt in range(NT)
            ]
            for ko in range(KO):
                for nt in range(NT):
                    nc.tensor.matmul(
                        pss[nt][:],
                        xbf[:, ko, :],
                        wq[:, ko, nt * 512 : (nt + 1) * 512],
                        start=(ko == 0),
                        stop=(ko == KO - 1),
                    )
            for nt in range(NT):
                ot = outp.tile([P, 512], F32, tag="ot")
                nc.vector.tensor_copy(ot[:], pss[nt][:])
                nc.sync.dma_start(out_r[:, mt, nt * 512 : (nt + 1) * 512], ot[:])

    nc.compile()
    return nc


@functools.lru_cache(maxsize=1)
def _built():
    return _build()


def kernel(x, weight, _trace=False, **_trace_kwargs):
    global LAST_RESULTS
    x = np.ascontiguousarray(np.asarray(x, dtype=np.float32).reshape(M, K))
    w = np.asarray(weight, dtype=np.float32)
    assert w.shape == (N, K)

    xt = np.ascontiguousarray(x.T)            # [K, M]
    wt = np.ascontiguousarray(w.T)            # [K, N]
    in_maps = [
        {
            "xt": xt,
            "wt": np.ascontiguousarray(wt[:, c * NS : (c + 1) * NS]),
        }
        for c in range(N_CORES)
    ]

    nc = _built()
    res = run_bass_kernel_spmd(
        nc, in_maps, core_ids=list(range(N_CORES)), trace=_trace, **_trace_kwargs
    )
    LAST_RESULTS = res
    out = np.concatenate(
        [res.results[c]["out"] for c in range(N_CORES)], axis=1
    )  # [M, N]
    return out.reshape(B, S, N)
